# revision 17
# baseline (speedup 1.0000x reference)
"""AFNO1D block (rfft -> block-diag complex MLP w/ GELU -> irfft -> +x) on 8 TRN2 cores.

Strategy:
  - Shard over C (8 channels -> 8 cores), weights replicated. No collectives.
  - rfft/irfft along L=2048 computed as dense DFT matmuls in bf16
    (frequencies padded 1025 -> 1152 = 9*128; inverse matrix zeroes the pad,
    so no DC/Nyquist special cases anywhere).
  - Forward DFT keeps p on PSUM partitions (x is the stationary operand), so
    the block-diagonal MLP (contracting within 64-wide p-blocks, packed as
    128x128 block-diagonal weight matrices) follows with no data movement.
  - Corner turn (p-part -> freq-part) before the inverse DFT is done with the
    DMA xbar transpose on bf16 [128,128] tiles.
  - GELU + b1, and the b2 add, are fused into the PSUM drains on ScalarE.
  - Residual `+ x` is an fp32 VectorE add of the original input.
"""

import numpy as np
import ml_dtypes
from contextlib import ExitStack

B, L, P = 4, 2048, 512
C = 8
NB, BS = 8, 64
LF = 1025
FPAD = 1152          # 9 * 128 padded frequency grid
NFT = FPAD // 128    # 9
FCHUNKS = [(0, 384), (384, 384), (768, 257)]  # fwd/L1 cover exactly [0,1025)
NLC = L // 128       # 16 l-chunks
NPC = P // 128       # 4 p-chunks

_BF16 = ml_dtypes.bfloat16


def _dft_consts():
    l = np.arange(L, dtype=np.float64)[:, None]
    f = np.arange(FPAD, dtype=np.float64)[None, :]
    s = 1.0 / np.sqrt(L)
    ang = 2.0 * np.pi * l * f / L
    fcos = (np.cos(ang) * s).astype(_BF16)
    fsin = (-np.sin(ang) * s).astype(_BF16)

    k = np.arange(FPAD, dtype=np.float64)[:, None]
    ck = np.where((k == 0) | (k == 1024), 1.0, 2.0)
    ck[k >= LF] = 0.0
    ang2 = 2.0 * np.pi * k * np.arange(L, dtype=np.float64)[None, :] / L
    iar = (ck * np.cos(ang2) * s).astype(_BF16)
    iai = -ck * np.sin(ang2) * s
    iai[0, :] = 0.0
    iai[1024, :] = 0.0   # irfft drops imag of DC/Nyquist
    iai = iai.astype(_BF16)
    return fcos, fsin, iar, iai


def _mlp_consts(w1, b1, w2, b2):
    # Block-diagonal 128x128 lhsT matrices: [input-unit j (partition), output-unit i]
    wm = np.zeros((2, NPC, 3, 128, 128), np.float32)
    for li, w in enumerate([w1, w2]):
        for pc in range(NPC):
            for t in range(2):
                n = 2 * pc + t
                sl = slice(64 * t, 64 * t + 64)
                wm[li, pc, 0][sl, sl] = w[0][n]          # real
                wm[li, pc, 1][sl, sl] = w[1][n]          # imag
                wm[li, pc, 2][sl, sl] = -w[1][n]         # -imag
    wmlp = np.ascontiguousarray(wm.reshape(2 * NPC * 3, 128, 128)).astype(_BF16)

    bias = np.zeros((NPC, 128, 4), np.float32)
    for pc in range(NPC):
        for t in range(2):
            n = 2 * pc + t
            sl = slice(64 * t, 64 * t + 64)
            bias[pc, sl, 0] = b1[0][n]
            bias[pc, sl, 1] = b1[1][n]
            bias[pc, sl, 2] = b2[0][n]
            bias[pc, sl, 3] = b2[1][n]
    return wmlp, bias


def _build_nc():
    import concourse.bacc as bacc
    import concourse.bass as bass
    import concourse.mybir as mybir
    import concourse.tile as tile

    dt = mybir.dt
    AF = mybir.ActivationFunctionType
    nc = bacc.Bacc("TRN2", target_bir_lowering=False, debug=False, num_devices=C)

    x_d = nc.declare_dram_parameter("x", [B, L, P], dt.float32, isOutput=False)
    fcos_d = nc.declare_dram_parameter("fcos", [L, FPAD], dt.bfloat16, isOutput=False)
    fsin_d = nc.declare_dram_parameter("fsin", [L, FPAD], dt.bfloat16, isOutput=False)
    iar_d = nc.declare_dram_parameter("iar", [FPAD, L], dt.bfloat16, isOutput=False)
    iai_d = nc.declare_dram_parameter("iai", [FPAD, L], dt.bfloat16, isOutput=False)
    wm_d = nc.declare_dram_parameter("wmlp", [24, 128, 128], dt.bfloat16, isOutput=False)
    bia_d = nc.declare_dram_parameter("bias", [NPC, 128, 4], dt.float32, isOutput=False)
    b2t_d = nc.declare_dram_parameter("b2t", [2, P], dt.float32, isOutput=False)
    out_d = nc.declare_dram_parameter("out", [B, L, P], dt.float32, isOutput=True)

    with tile.TileContext(nc) as tc, ExitStack() as ctx:
        pool = lambda name, bufs: ctx.enter_context(tc.tile_pool(name=name, bufs=bufs))
        consts = pool("consts", 1)
        xf32 = pool("xf32", 2)
        xbf = pool("xbf", 6)
        fmat = pool("fmat", 2)
        xdr = pool("xdr", 4)
        o1p = pool("o1p", 9)
        o2t = pool("o2t", 36)
        imat = pool("imat", 3)
        outp = pool("outp", 4)
        xres = pool("xres", 4)
        ps = ctx.enter_context(tc.tile_pool(name="ps", bufs=8, space="PSUM"))

        wm_sb = consts.tile([128, 24, 128], dt.bfloat16)
        nc.gpsimd.dma_start(out=wm_sb, in_=wm_d[:].rearrange("n j i -> j n i"))
        bia_sb = consts.tile([128, NPC, 4], dt.float32)
        nc.gpsimd.dma_start(out=bia_sb, in_=bia_d[:].rearrange("c p k -> p c k"))
        b2t_sb = consts.tile([128, 2, P], dt.float32)
        nc.gpsimd.dma_start(
            out=b2t_sb,
            in_=bass.AP(tensor=b2t_d, offset=0, ap=[[0, 128], [P, 2], [1, P]]),
        )

        def wtile(layer, pc, kind):
            return wm_sb[:, (layer * NPC + pc) * 3 + kind, :]

        def btile(pc, kind):
            return bia_sb[:, pc, kind : kind + 1]

        o2t_tiles = {}

        def fwd_mlp_batch(b):
            """Forward DFT + MLP for batch b; fills o2t_tiles[(b, ft, ri)]."""
            xbfs = []
            for q in range(4):
                xf = xf32.tile([128, 4, P], dt.float32, tag="xf32")
                nc.sync.dma_start(
                    out=xf,
                    in_=x_d[b].rearrange("(lc l) p -> l lc p", l=128)[:, q * 4 : q * 4 + 4, :],
                )
                xb = xbf.tile([128, 4, P], dt.bfloat16, tag="xbf")
                nc.vector.tensor_copy(out=xb, in_=xf)
                xbfs.extend(xb[:, j, :] for j in range(4))

            o1sb = {}
            for pc in range(NPC):
                for ri in range(2):
                    o1sb[(pc, ri)] = o1p.tile(
                        [128, FPAD], dt.bfloat16, tag="o1sb", name=f"o1sb_{b}_{pc}_{ri}"
                    )
                    nc.vector.memset(o1sb[(pc, ri)][:, LF:], 0.0)

            # ---- phase 1: forward DFT + MLP layer 1 (+GELU), p on partitions ----
            for f0, fw in FCHUNKS:
                fmc = fmat.tile([128, NLC, fw], dt.bfloat16, tag="fmatc")
                fms = fmat.tile([128, NLC, fw], dt.bfloat16, tag="fmats")
                for q in range(4):
                    qs = slice(q * 4, q * 4 + 4)
                    nc.scalar.dma_start(
                        out=fmc[:, qs, :],
                        in_=fcos_d[:].rearrange("(lc l) f -> l lc f", l=128)[:, qs, f0 : f0 + fw],
                    )
                    nc.scalar.dma_start(
                        out=fms[:, qs, :],
                        in_=fsin_d[:].rearrange("(lc l) f -> l lc f", l=128)[:, qs, f0 : f0 + fw],
                    )
                cs = [(fmc[:, lc, :], fms[:, lc, :]) for lc in range(NLC)]
                for pc in range(NPC):
                    ps_r = ps.tile([128, fw], dt.float32, tag="ps")
                    ps_i = ps.tile([128, fw], dt.float32, tag="ps")
                    for lc in range(NLC):
                        xs = xbfs[lc][:, pc * 128 : (pc + 1) * 128]
                        nc.tensor.matmul(
                            ps_r, lhsT=xs, rhs=cs[lc][0],
                            start=(lc == 0), stop=(lc == NLC - 1),
                        )
                        nc.tensor.matmul(
                            ps_i, lhsT=xs, rhs=cs[lc][1],
                            start=(lc == 0), stop=(lc == NLC - 1),
                        )
                    xr_sb = xdr.tile([128, fw], dt.bfloat16, tag="xr")
                    nc.scalar.activation(out=xr_sb, in_=ps_r, func=AF.Copy)
                    xi_sb = xdr.tile([128, fw], dt.bfloat16, tag="xi")
                    nc.vector.tensor_copy(out=xi_sb, in_=ps_i)

                    p1r = ps.tile([128, fw], dt.float32, tag="ps")
                    nc.tensor.matmul(p1r, lhsT=wtile(0, pc, 0), rhs=xr_sb, start=True, stop=False)
                    nc.tensor.matmul(p1r, lhsT=wtile(0, pc, 2), rhs=xi_sb, start=False, stop=True)
                    p1i = ps.tile([128, fw], dt.float32, tag="ps")
                    nc.tensor.matmul(p1i, lhsT=wtile(0, pc, 1), rhs=xr_sb, start=True, stop=False)
                    nc.tensor.matmul(p1i, lhsT=wtile(0, pc, 0), rhs=xi_sb, start=False, stop=True)

                    fsl = slice(f0, f0 + fw)
                    nc.scalar.activation(
                        out=o1sb[(pc, 0)][:, fsl], in_=p1r, func=AF.Gelu,
                        bias=btile(pc, 0), scale=1.0,
                    )
                    nc.scalar.activation(
                        out=o1sb[(pc, 1)][:, fsl], in_=p1i, func=AF.Gelu,
                        bias=btile(pc, 1), scale=1.0,
                    )

            # ---- phase 2: MLP layer 2 with o1 as stationary operand ----
            # out psum = o1_slice.T @ W2bd -> [f-chunk (partitions), (n,i')] : the
            # corner turn to freq-partitions happens inside the matmul.
            for ft in range(NFT):
                q_r = ps.tile([128, P], dt.float32, tag="ps")
                q_i = ps.tile([128, P], dt.float32, tag="ps")
                for pc in range(NPC):
                    fs = slice(ft * 128, (ft + 1) * 128)
                    os = slice(pc * 128, (pc + 1) * 128)
                    o1r_sl = o1sb[(pc, 0)][:, fs]
                    o1i_sl = o1sb[(pc, 1)][:, fs]
                    nc.tensor.matmul(q_r[:, os], lhsT=o1r_sl, rhs=wtile(1, pc, 0), start=True, stop=False)
                    nc.tensor.matmul(q_i[:, os], lhsT=o1r_sl, rhs=wtile(1, pc, 1), start=True, stop=False)
                    nc.tensor.matmul(q_r[:, os], lhsT=o1i_sl, rhs=wtile(1, pc, 2), start=False, stop=True)
                    nc.tensor.matmul(q_i[:, os], lhsT=o1i_sl, rhs=wtile(1, pc, 0), start=False, stop=True)
                for ri, q in ((0, q_r), (1, q_i)):
                    o2t_tiles[(b, ft, ri)] = o2t.tile(
                        [128, P], dt.bfloat16, tag="o2t", name=f"o2t_{b}_{ft}_{ri}"
                    )
                    nc.vector.tensor_add(
                        out=o2t_tiles[(b, ft, ri)], in0=q, in1=b2t_sb[:, ri, :]
                    )

        def inv_batches(bs_):
            """Inverse DFT + residual + store for batches bs_ (o2t tiles ready)."""
            for lt in range(NLC):
                iat_r = imat.tile([128, NFT, 128], dt.bfloat16, tag="iart")
                nc.sync.dma_start(
                    out=iat_r,
                    in_=iar_d[:].rearrange("(kc k) l -> k kc l", k=128)[
                        :, :, lt * 128 : (lt + 1) * 128
                    ],
                )
                iat_i = imat.tile([128, NFT, 128], dt.bfloat16, tag="iait")
                nc.sync.dma_start(
                    out=iat_i,
                    in_=iai_d[:].rearrange("(kc k) l -> k kc l", k=128)[
                        :, :, lt * 128 : (lt + 1) * 128
                    ],
                )
                ia = [(iat_r[:, kc, :], iat_i[:, kc, :]) for kc in range(NFT)]
                for b in bs_:
                    pso = ps.tile([128, P], dt.float32, tag="ps")
                    for kc in range(NFT):
                        nc.tensor.matmul(
                            pso, lhsT=ia[kc][0], rhs=o2t_tiles[(b, kc, 0)],
                            start=(kc == 0), stop=(kc == NFT - 1),
                        )
                        if kc < NFT - 1:  # IAi rows for k>=1024 are all zero
                            nc.tensor.matmul(
                                pso, lhsT=ia[kc][1], rhs=o2t_tiles[(b, kc, 1)],
                                start=False, stop=False,
                            )
                    xr = xres.tile([128, P], dt.float32, tag="xres")
                    nc.scalar.dma_start(out=xr, in_=x_d[b, lt * 128 : (lt + 1) * 128, :])
                    ob = outp.tile([128, P], dt.float32, tag="outp")
                    nc.vector.tensor_add(out=ob, in0=pso, in1=xr)
                    nc.scalar.dma_start(out=out_d[b, lt * 128 : (lt + 1) * 128, :], in_=ob)

        fwd_mlp_batch(0)
        fwd_mlp_batch(1)
        inv_batches([0, 1])
        fwd_mlp_batch(2)
        fwd_mlp_batch(3)
        inv_batches([2, 3])

    nc.compile()
    return nc


_NC_CACHE = None
LAST_EXEC_NS = None


def _ensure_hook_shim():
    # bass_utils imports antenv.axon_hooks when trace=True; some images lack
    # it. Pre-install a null shim so tracing degrades instead of crashing.
    import sys, types

    if "antenv.axon_hooks" not in sys.modules:
        m = types.ModuleType("antenv.axon_hooks")
        holder = [None]
        m.set_axon_ntff_profile_hook = lambda h: holder.__setitem__(0, h)
        m.get_axon_ntff_profile_hook = lambda: holder[0]
        try:
            import antenv.axon_hooks  # noqa: F401  # real module exists
        except ImportError:
            sys.modules["antenv.axon_hooks"] = m


def kernel(**inputs):
    global _NC_CACHE, LAST_EXEC_NS
    _ensure_hook_shim()
    from concourse.bass_utils import run_bass_kernel_spmd

    x = np.asarray(inputs["x"], dtype=np.float32)
    w1 = np.asarray(inputs["w1"], dtype=np.float32)
    b1 = np.asarray(inputs["b1"], dtype=np.float32)
    w2 = np.asarray(inputs["w2"], dtype=np.float32)
    b2 = np.asarray(inputs["b2"], dtype=np.float32)

    fcos, fsin, iar, iai = _dft_consts()
    wmlp, bias = _mlp_consts(w1, b1, w2, b2)

    if _NC_CACHE is None:
        _NC_CACHE = _build_nc()
    nc = _NC_CACHE

    in_maps = []
    for c in range(C):
        in_maps.append(
            dict(
                x=np.ascontiguousarray(x[:, :, :, c]),
                fcos=fcos, fsin=fsin, iar=iar, iai=iai,
                wmlp=wmlp, bias=bias,
                b2t=np.ascontiguousarray(b2.reshape(2, P)).astype(np.float32),
            )
        )

    import os
    res = run_bass_kernel_spmd(
        nc, in_maps, core_ids=list(range(C)),
        trace=bool(os.environ.get("BASS_TRACE")),
    )
    LAST_EXEC_NS = getattr(res, "exec_time_ns", None)

    out = np.empty((B, L, P, C), np.float32)
    for c in range(C):
        out[:, :, :, c] = res.results[c]["out"]
    return out
